# revision 18
# baseline (speedup 1.0000x reference)
"""Distributed GQA attention block (dense transformer) on 8 TRN2 NeuronCores.

Reference computation (per problem):
  xq = x @ wq.T ; xk = x @ wk.T ; xv = x @ wv.T      (torch-Linear style)
  RoPE (interleaved pairs) on xq, xk
  GQA causal attention (32 q heads, 8 kv heads, head_dim 128, seq 2048)
  out = attn_out @ wo.T

Sharding: tensor-parallel over heads. Core c gets q heads [4c, 4c+4) (rows
512c:512c+512 of wq) and kv head c. For the output projection, each core
AllGathers the (feature-major) attention output y of a token chunk from all
cores, then computes its own 512 OUTPUT columns of `out` with a local slice
of wo (rows 512c:512c+512 of wo); the host concatenates the column shards.
This replaces a ReduceScatter of full-size partials: half the wire bytes at
the cheaper (copy, not reduce) rate, pipelined per chunk so no collective
tail remains.

Device pipeline per core (matmuls bf16, f32 accumulation, PE-friendly):
  1. Projection with WEIGHTS stationary and xT moving: qT/kT come out of
     PSUM directly in [feature, token] layout (no PE transposes). RoPE is
     applied in [f, t] layout: rot(x) = x*CF + (Pm.T @ x)*SF where Pm is a
     sign-folded pair-swap permutation done as one 512-row matmul per
     block and CF/SF are precomputed [128, SEQ] tables. vT is transposed
     to natural [token, dv] via 4 PE transposes per chunk. k/v projected
     first so attention (and the chunk's AllGather) starts early.
  2. Flash-style causal attention per (chunk, head) with 128-granular
     causal narrowing (diagonal j-tiles only compute q >= kv columns).
     exp on ACT; softmax denominators accumulated on DVE in f32 and
     reduced with a single ones-matmul; reciprocal via the fast
     Newton-iteration DVE op; 1/l broadcast via gpsimd partition_broadcast.
  3. Per chunk: y (feature-major) -> DRAM -> AllGather -> wo matmul of the
     core's output-column shard -> DMA to out. wo for chunk c is emitted
     after attention of chunk c+1 so the AllGather is hidden by compute;
     the gathered yT is consumed in token-halves so loads pipeline. The
     last chunk's AllGather is split into two token-halves to shrink the
     serial tail.
"""
import sys

sys.path.insert(0, "/opt/trn_rl_repo")

import numpy as np
import ml_dtypes

from concourse import bass, bacc, tile, mybir
from concourse.bass_utils import run_bass_kernel_spmd

N_CORES = 8
DIM = 4096
N_HEADS = 32
HEAD_DIM = 128
SEQ = 2048
ROPE_THETA = 10000.0

HQ = N_HEADS // N_CORES          # 4 local q heads
FQ = HQ * HEAD_DIM               # 512 q features per core
KT = DIM // 128                  # 32 contraction tiles
TT = SEQ // 128                  # 16 token tiles
NCH = 4                          # token chunks
CHUNK = SEQ // NCH               # 512
HALF = CHUNK // 2                # 256
SCALE = 1.0 / float(np.sqrt(HEAD_DIM))

F32 = mybir.dt.float32
BF16 = mybir.dt.bfloat16
AL = mybir.AluOpType


def build_nc():
    nc = bacc.Bacc("TRN2", target_bir_lowering=False, debug=False,
                   num_devices=N_CORES)

    # ---- external inputs (host pre-casts to bf16, pre-transposes weights) --
    x_ext = nc.dram_tensor("xT", [DIM, SEQ], BF16, kind="ExternalInput")
    wqT_ext = nc.dram_tensor("wqT", [DIM, FQ], BF16, kind="ExternalInput")
    wkvT_ext = nc.dram_tensor("wkvT", [DIM, 256], BF16, kind="ExternalInput")
    woT_ext = nc.dram_tensor("woT", [DIM, FQ], BF16, kind="ExternalInput")
    cf_ext = nc.dram_tensor("cf", [128, SEQ], BF16, kind="ExternalInput")
    sf_ext = nc.dram_tensor("sf", [128, SEQ], BF16, kind="ExternalInput")
    pm_ext = nc.dram_tensor("pm", [128, 128], BF16, kind="ExternalInput")
    tri_ext = nc.dram_tensor("tri", [128, 128], BF16, kind="ExternalInput")
    id_ext = nc.dram_tensor("ident", [128, 128], BF16, kind="ExternalInput")

    out_ext = nc.dram_tensor("out", [SEQ, FQ], BF16, kind="ExternalOutput")

    # ---- internal DRAM: per-chunk y (two head-group pieces) + gathers ----
    # piece p of chunk c holds local heads {2p, 2p+1}; the AllGather of a
    # piece yields 2048 of the 4096 y-features (host reorders woT rows to
    # match), so each piece can fire as soon as two heads are done and wo
    # accumulates across the two gathered pieces.
    y_dram = [[nc.dram_tensor(f"ych{c}p{p}", [256, CHUNK], BF16)
               for p in range(2)] for c in range(NCH)]
    ag_dram = [[nc.dram_tensor(f"agch{c}p{p}", [DIM // 2, CHUNK], BF16,
                               addr_space="Shared")
                for p in range(2)] for c in range(NCH)]

    with tile.TileContext(nc) as tc:
        pers_cm = tc.tile_pool(name="pers", bufs=1)
        pers = pers_cm.__enter__()
        wqT = pers.tile([128, KT, FQ], BF16, tag="wqT")      # [d, kt, f]
        woT = pers.tile([128, KT, FQ], BF16, tag="woT")      # [yf, kt, of]
        cf = pers.tile([128, SEQ], BF16, tag="cf")           # rope cos [f, t]
        sft = pers.tile([128, SEQ], BF16, tag="sf")          # rope sin [f, t]
        pm = pers.tile([128, 128], BF16, tag="pm")           # signed pairswap
        tri = pers.tile([128, 128], BF16, tag="tri")         # causal 128-blk
        ident = pers.tile([128, 128], BF16, tag="ident")
        kTt = pers.tile([128, SEQ], BF16, tag="kTt")         # [d, t]
        vS = pers.tile([128, TT, HEAD_DIM], BF16, tag="vS")  # [t_loc, tt, dv]
        ones_b = pers.tile([128, 1], BF16, tag="ones_b")

        nc.any.memset(ones_b[:, :], 1.0)

        with tc.tile_pool(name="ps_a", bufs=2, space="PSUM") as pa, \
             tc.tile_pool(name="ps_sc", bufs=3, space="PSUM") as psc, \
             tc.tile_pool(name="ps_o", bufs=2, space="PSUM") as po, \
             tc.tile_pool(name="ps_m", bufs=1, space="PSUM") as pmp, \
             tc.tile_pool(name="xp", bufs=33) as xp, \
             tc.tile_pool(name="wkvp", bufs=8) as wkvp, \
             tc.tile_pool(name="qtp", bufs=2) as qtp, \
             tc.tile_pool(name="rp", bufs=2) as rp, \
             tc.tile_pool(name="sm", bufs=4) as smp, \
             tc.tile_pool(name="exp", bufs=6) as exp_, \
             tc.tile_pool(name="esp", bufs=2) as esp, \
             tc.tile_pool(name="yp", bufs=1) as yp, \
             tc.tile_pool(name="ytp", bufs=2) as ytp, \
             tc.tile_pool(name="owp", bufs=3) as owp:

            def rope_block(src_sb, dst_ap, tslice):
                """dst = src*CF + (Pm.T @ src)*SF  on a [128, CHUNK] block."""
                ps_sw = psc.tile([128, CHUNK], F32, tag="sc")
                nc.tensor.matmul(ps_sw[:, :], pm[:, :], src_sb[:, :],
                                 start=True, stop=True)
                t1 = rp.tile([128, CHUNK], BF16, tag="t1")
                t2 = rp.tile([128, CHUNK], BF16, tag="t2")
                nc.vector.tensor_tensor(out=t1[:, :], in0=src_sb[:, :],
                                        in1=cf[:, tslice], op=AL.mult)
                nc.vector.tensor_tensor(out=t2[:, :], in0=ps_sw[:, :],
                                        in1=sft[:, tslice], op=AL.mult)
                nc.vector.tensor_tensor(out=dst_ap, in0=t1[:, :],
                                        in1=t2[:, :], op=AL.add)

            def emit_wo(c):
                """wo for chunk c from the two gathered head-group pieces.

                Each [128, CHUNK] output accumulates kt 0-15 from piece 0 and
                kt 16-31 from piece 1; the piece-0 half-groups are emitted
                first (piece 0's AllGather completes ~25us before piece 1's),
                two tl groups at a time to fit the 2-bank acc pool. Pieces
                load in 1 MB sub-loads so the first kt tiles land sooner.
                """
                yts = []
                for p in range(2):
                    yt = ytp.tile([128, KT // 2, CHUNK], BF16, tag="yt")
                    for sub in range(2):
                        nc.sync.dma_start(
                            out=yt[:, 8 * sub:8 * (sub + 1), :],
                            in_=ag_dram[c][p].ap()[1024 * sub:
                                                   1024 * (sub + 1), :]
                            .rearrange("(kt p) t -> p kt t", p=128))
                    yts.append(yt)
                for pair in ((0, 1), (2, 3)):
                    pws = []
                    for tl in pair:
                        ps_w = pa.tile([128, CHUNK], F32, tag="acc")
                        for kt in range(16):
                            nc.tensor.matmul(
                                ps_w[:, :],
                                yts[0][:, kt, 128 * tl:128 * (tl + 1)],
                                woT[:, kt, :],
                                start=(kt == 0), stop=False)
                        pws.append(ps_w)
                    for ps_w, tl in zip(pws, pair):
                        for kt in range(16, KT):
                            nc.tensor.matmul(
                                ps_w[:, :],
                                yts[1][:, kt - 16, 128 * tl:128 * (tl + 1)],
                                woT[:, kt, :],
                                start=False, stop=(kt == KT - 1))
                    for ps_w, tl in zip(pws, pair):
                        ow = owp.tile([128, CHUNK], BF16, tag="ow")
                        nc.vector.tensor_copy(out=ow[:, :], in_=ps_w[:, :])
                        r0 = CHUNK * c + 128 * tl
                        nc.gpsimd.dma_start(out=out_ext[r0:r0 + 128, :],
                                            in_=ow[:, :])

            for c in range(NCH):
                tsl = slice(CHUNK * c, CHUNK * (c + 1))
                # ---- loads spread across engine queues (issue-rate bound
                # at startup): x on sync, wkv on gpsimd, wq on scalar ----
                xts = []
                wkvs = []
                for k in range(KT):
                    wkv = wkvp.tile([128, 256], BF16, tag="wkv")
                    nc.gpsimd.dma_start(out=wkv[:, :],
                                        in_=wkvT_ext[128 * k:128 * (k + 1), :])
                    wkvs.append(wkv)
                    xt = xp.tile([128, CHUNK], BF16, tag="xT")
                    nc.sync.dma_start(out=xt[:, :], in_=x_ext[
                        128 * k:128 * (k + 1), tsl])
                    xts.append(xt)
                if c == 0:
                    for k in range(KT):
                        nc.scalar.dma_start(
                            out=wqT[:, k, :],
                            in_=wqT_ext[128 * k:128 * (k + 1), :])
                    nc.scalar.dma_start(out=cf[:, :], in_=cf_ext[:, :])
                    nc.scalar.dma_start(out=sft[:, :], in_=sf_ext[:, :])
                    nc.scalar.dma_start(out=pm[:, :], in_=pm_ext[:, :])
                    nc.scalar.dma_start(out=tri[:, :], in_=tri_ext[:, :])
                    nc.scalar.dma_start(out=ident[:, :], in_=id_ext[:, :])
                    for k in range(KT):
                        nc.scalar.dma_start(
                            out=woT[:, k, :],
                            in_=woT_ext[128 * k:128 * (k + 1), :])

                # ---- projection: weights stationary, xT moving; k/v first --
                ps_k = pa.tile([128, CHUNK], F32, tag="acc")
                for k in range(KT):
                    nc.tensor.matmul(ps_k[:, :], wkvs[k][:, 0:128],
                                     xts[k][:, :],
                                     start=(k == 0), stop=(k == KT - 1))
                kb = rp.tile([128, CHUNK], BF16, tag="qb")
                nc.vector.tensor_copy(out=kb[:, :], in_=ps_k[:, :])
                rope_block(kb, kTt[:, tsl], tsl)
                # v -> vT [dv, t], then transpose to natural [t, dv]
                ps_v = pa.tile([128, CHUNK], F32, tag="acc")
                for k in range(KT):
                    nc.tensor.matmul(ps_v[:, :], wkvs[k][:, 128:256],
                                     xts[k][:, :],
                                     start=(k == 0), stop=(k == KT - 1))
                vtb = rp.tile([128, CHUNK], BF16, tag="qb")
                nc.vector.tensor_copy(out=vtb[:, :], in_=ps_v[:, :])
                for tl in range(4):
                    ps_tr = pmp.tile([128, 128], BF16, tag="m")
                    nc.tensor.transpose(ps_tr[:, :],
                                        vtb[:, 128 * tl:128 * (tl + 1)],
                                        ident[:, :])
                    nc.vector.tensor_copy(out=vS[:, 4 * c + tl, :],
                                          in_=ps_tr[:, :])
                # q blocks
                qT = qtp.tile([128, HQ, CHUNK], BF16, tag="qT")
                for b in range(HQ):
                    ps_q = pa.tile([128, CHUNK], F32, tag="acc")
                    for k in range(KT):
                        nc.tensor.matmul(ps_q[:, :],
                                         wqT[:, k, 128 * b:128 * (b + 1)],
                                         xts[k][:, :],
                                         start=(k == 0), stop=(k == KT - 1))
                    qb = rp.tile([128, CHUNK], BF16, tag="qb")
                    nc.vector.tensor_copy(out=qb[:, :], in_=ps_q[:, :])
                    rope_block(qb, qT[:, b, :], tsl)

                # ---- attention for chunk c ----
                y_sb = yp.tile([128, HQ, CHUNK], BF16, tag="y")
                njt = 4 * (c + 1)
                for h in range(HQ):
                    ps_o = po.tile([128, CHUNK], F32, tag="o")
                    exsum = esp.tile([128, CHUNK], F32, tag="es")
                    for jt in range(njt):
                        p = jt - 4 * c
                        lo = 128 * p if p > 0 else 0
                        ps_s = psc.tile([128, CHUNK], F32, tag="sc")
                        nc.tensor.matmul(ps_s[:, lo:CHUNK],
                                         kTt[:, 128 * jt:128 * (jt + 1)],
                                         qT[:, h, lo:CHUNK],
                                         start=True, stop=True)
                        exf = exp_.tile([128, CHUNK], BF16, tag="ex")
                        nc.scalar.activation(
                            out=exf[:, lo:CHUNK], in_=ps_s[:, lo:CHUNK],
                            func=mybir.ActivationFunctionType.Exp,
                            scale=SCALE)
                        if p >= 0:
                            nc.vector.tensor_tensor(
                                out=exf[:, lo:lo + 128],
                                in0=exf[:, lo:lo + 128],
                                in1=tri[:, :], op=AL.mult)
                        nc.tensor.matmul(ps_o[:, lo:CHUNK], vS[:, jt, :],
                                         exf[:, lo:CHUNK],
                                         start=(jt == 0),
                                         stop=(jt == njt - 1))
                        if jt == 0:
                            nc.vector.tensor_copy(out=exsum[:, :],
                                                  in_=exf[:, :])
                        else:
                            nc.vector.tensor_tensor(
                                out=exsum[:, lo:CHUNK],
                                in0=exsum[:, lo:CHUNK],
                                in1=exf[:, lo:CHUNK], op=AL.add)
                    # softmax denominator -> broadcast 1/l -> normalize
                    esb = rp.tile([128, CHUNK], BF16, tag="esb")
                    nc.vector.tensor_copy(out=esb[:, :], in_=exsum[:, :])
                    ps_l = pmp.tile([1, CHUNK], F32, tag="m")
                    nc.tensor.matmul(ps_l[:, :], ones_b[:, :], esb[:, :],
                                     start=True, stop=True)
                    rr = smp.tile([1, CHUNK], F32, tag="rr")
                    nc.vector.reciprocal_approx_fast(out=rr[:, :],
                                                     in_=ps_l[:, :])
                    bc = rp.tile([128, CHUNK], F32, tag="bc")
                    nc.gpsimd.partition_broadcast(bc[:, :], rr[:, :])
                    nc.vector.tensor_tensor(out=y_sb[:, h, :],
                                            in0=ps_o[:, :], in1=bc[:, :],
                                            op=AL.mult)
                    # ---- piece done after heads 1 and 3: DRAM + AllGather
                    if h % 2 == 1:
                        p = h // 2
                        for hh in (h - 1, h):
                            nc.gpsimd.dma_start(
                                out=y_dram[c][p][128 * (hh - 2 * p):
                                                 128 * (hh - 2 * p + 1), :],
                                in_=y_sb[:, hh, :])
                        nc.gpsimd.collective_compute(
                            "AllGather", AL.bypass,
                            replica_groups=[list(range(N_CORES))],
                            ins=[y_dram[c][p].ap().opt()],
                            outs=[ag_dram[c][p].ap().opt()])
                    # previous chunk's wo fills the rest of this attention
                    if h == 1 and c >= 1:
                        emit_wo(c - 1)
            emit_wo(NCH - 1)

        pers_cm.__exit__(None, None, None)

    nc.finalize()
    return nc


_NC_CACHE = None


def _get_nc():
    global _NC_CACHE
    if _NC_CACHE is None:
        _NC_CACHE = build_nc()
    return _NC_CACHE


def _host_constants():
    bf = ml_dtypes.bfloat16
    m = np.arange(64, dtype=np.float64)
    freqs = 1.0 / (ROPE_THETA ** (2.0 * m / HEAD_DIM))
    t = np.arange(SEQ, dtype=np.float64)
    ang = np.outer(freqs, t)                                 # [64, SEQ]
    cfv = np.repeat(np.cos(ang), 2, axis=0).astype(bf)       # [128, SEQ]
    sfv = np.repeat(np.sin(ang), 2, axis=0).astype(bf)
    # signed pair swap: out[2m] = -in[2m+1], out[2m+1] = +in[2m]
    # out = Pm.T @ in  ->  Pm[2m+1, 2m] = -1 ; Pm[2m, 2m+1] = +1
    pmv = np.zeros((128, 128), np.float32)
    idx = np.arange(0, 128, 2)
    pmv[idx + 1, idx] = -1.0
    pmv[idx, idx + 1] = 1.0
    pmv = pmv.astype(bf)
    j = np.arange(128)[:, None]
    i = np.arange(128)[None, :]
    triv = (j <= i).astype(np.float32).astype(bf)
    identv = np.eye(128, dtype=bf)
    return cfv, sfv, pmv, triv, identv


def _make_in_maps(x, wq, wk, wv, wo):
    cfv, sfv, pmv, triv, identv = _host_constants()
    bf = ml_dtypes.bfloat16
    xT2 = np.ascontiguousarray(x.reshape(SEQ, DIM).astype(bf).T)
    wqT = np.ascontiguousarray(wq.T.astype(bf))              # [DIM, 4096]
    wkT = wk.T.astype(bf)                                    # [DIM, 1024]
    wvT = wv.T.astype(bf)
    in_maps = []
    for c in range(N_CORES):
        wkvT = np.concatenate([wkT[:, HEAD_DIM * c:HEAD_DIM * (c + 1)],
                               wvT[:, HEAD_DIM * c:HEAD_DIM * (c + 1)]],
                              axis=1)
        # rows of wo for OUR output columns, transposed: [DIM(yfeat), FQ].
        # The AllGather pieces deliver y-features in order
        # [rank0 h01, rank1 h01, ..., rank7 h01, rank0 h23, ...]:
        # reorder woT rows to match.
        woTc = wo[FQ * c:FQ * (c + 1), :].T.astype(bf)       # [DIM, FQ]
        woTc = np.ascontiguousarray(
            woTc.reshape(N_CORES, 2, 256, FQ).transpose(1, 0, 2, 3)
                .reshape(DIM, FQ))
        in_maps.append({
            "xT": xT2,
            "wqT": np.ascontiguousarray(wqT[:, FQ * c:FQ * (c + 1)]),
            "wkvT": np.ascontiguousarray(wkvT),
            "woT": woTc,
            "cf": cfv, "sf": sfv, "pm": pmv, "tri": triv, "ident": identv,
        })
    return in_maps


def _assemble(results):
    # core c holds out[:, 512c:512c+512]
    cols = [np.asarray(results[c]["out"]).astype(np.float32)
            for c in range(N_CORES)]
    return np.concatenate(cols, axis=1).reshape(1, SEQ, DIM)


def run(inputs, trace=False, tmpdir=None):
    nc = _get_nc()
    in_maps = _make_in_maps(inputs["x"], inputs["wq"], inputs["wk"],
                            inputs["wv"], inputs["wo"])
    res = run_bass_kernel_spmd(nc, in_maps, list(range(N_CORES)),
                               trace=trace, tmpdir=tmpdir)
    return _assemble(res.results), res


def kernel(x, start_pos, wq, wk, wv, wo):
    out, _ = run({"x": np.asarray(x), "wq": np.asarray(wq),
                  "wk": np.asarray(wk), "wv": np.asarray(wv),
                  "wo": np.asarray(wo)})
    return out


if __name__ == "__main__":
    rng = np.random.default_rng(0)
    x = rng.standard_normal((1, SEQ, DIM)).astype(np.float32)
    wq = (rng.standard_normal((DIM, DIM)) * DIM ** -0.5).astype(np.float32)
    wk = (rng.standard_normal((1024, DIM)) * DIM ** -0.5).astype(np.float32)
    wv = (rng.standard_normal((1024, DIM)) * DIM ** -0.5).astype(np.float32)
    wo = (rng.standard_normal((DIM, DIM)) * DIM ** -0.5).astype(np.float32)
    out = kernel(x, 0, wq, wk, wv, wo)
    print(out.shape, out.dtype, np.abs(out).mean())


# revision 22
# speedup vs baseline: 1.1351x; 1.1351x over previous
"""Distributed GQA attention block (dense transformer) on 8 TRN2 NeuronCores.

Reference computation (per problem):
  xq = x @ wq.T ; xk = x @ wk.T ; xv = x @ wv.T      (torch-Linear style)
  RoPE (interleaved pairs) on xq, xk
  GQA causal attention (32 q heads, 8 kv heads, head_dim 128, seq 2048)
  out = attn_out @ wo.T

Sharding: tensor-parallel over heads. Core c gets q heads [4c, 4c+4) (rows
512c:512c+512 of wq) and kv head c. For the output projection, each core
AllGathers the (feature-major) attention output y of a token chunk from all
cores, then computes its own 512 OUTPUT columns of `out` with a local slice
of wo (rows 512c:512c+512 of wo); the host concatenates the column shards.
This replaces a ReduceScatter of full-size partials: half the wire bytes at
the cheaper (copy, not reduce) rate, pipelined per chunk so no collective
tail remains.

Device pipeline per core (matmuls bf16, f32 accumulation, PE-friendly):
  1. Projection with WEIGHTS stationary and xT moving: qT/kT come out of
     PSUM directly in [feature, token] layout (no PE transposes). RoPE is
     applied in [f, t] layout: rot(x) = x*CF + (Pm.T @ x)*SF where Pm is a
     sign-folded pair-swap permutation done as one 512-row matmul per
     block and CF/SF are precomputed [128, SEQ] tables. vT is transposed
     to natural [token, dv] via 4 PE transposes per chunk. k/v projected
     first so attention (and the chunk's AllGather) starts early.
  2. Flash-style causal attention per (chunk, head) with 128-granular
     causal narrowing (diagonal j-tiles only compute q >= kv columns).
     exp on ACT; softmax denominators accumulated on DVE in f32 and
     reduced with a single ones-matmul; reciprocal via the fast
     Newton-iteration DVE op; 1/l broadcast via gpsimd partition_broadcast.
  3. Per chunk: y (feature-major) -> DRAM -> AllGather -> wo matmul of the
     core's output-column shard -> DMA to out. wo for chunk c is emitted
     after attention of chunk c+1 so the AllGather is hidden by compute;
     the gathered yT is consumed in token-halves so loads pipeline. The
     last chunk's AllGather is split into two token-halves to shrink the
     serial tail.
"""
import sys

sys.path.insert(0, "/opt/trn_rl_repo")

import numpy as np
import ml_dtypes

from concourse import bass, bacc, tile, mybir
from concourse.bass_utils import run_bass_kernel_spmd

N_CORES = 8
DIM = 4096
N_HEADS = 32
HEAD_DIM = 128
SEQ = 2048
ROPE_THETA = 10000.0

HQ = N_HEADS // N_CORES          # 4 local q heads
FQ = HQ * HEAD_DIM               # 512 q features per core
KT = DIM // 128                  # 32 contraction tiles
TT = SEQ // 128                  # 16 token tiles
NCH = 4                          # token chunks
CHUNK = SEQ // NCH               # 512
HALF = CHUNK // 2                # 256
SCALE = 1.0 / float(np.sqrt(HEAD_DIM))

F32 = mybir.dt.float32
BF16 = mybir.dt.bfloat16
AL = mybir.AluOpType


def build_nc():
    nc = bacc.Bacc("TRN2", target_bir_lowering=False, debug=False,
                   num_devices=N_CORES)

    # ---- external inputs (host pre-casts to bf16, pre-transposes weights) --
    x_ext = nc.dram_tensor("xT", [DIM, SEQ], BF16, kind="ExternalInput")
    wqT_ext = nc.dram_tensor("wqT", [DIM, FQ], BF16, kind="ExternalInput")
    wkvT_ext = nc.dram_tensor("wkvT", [DIM, 256], BF16, kind="ExternalInput")
    woT_ext = nc.dram_tensor("woT", [DIM, FQ], BF16, kind="ExternalInput")
    cf_ext = nc.dram_tensor("cf", [128, SEQ], BF16, kind="ExternalInput")
    sf_ext = nc.dram_tensor("sf", [128, SEQ], BF16, kind="ExternalInput")
    pm_ext = nc.dram_tensor("pm", [128, 128], BF16, kind="ExternalInput")
    tri_ext = nc.dram_tensor("tri", [128, 128], BF16, kind="ExternalInput")
    id_ext = nc.dram_tensor("ident", [128, 128], BF16, kind="ExternalInput")

    out_ext = nc.dram_tensor("out", [SEQ, FQ], BF16, kind="ExternalOutput")

    # ---- internal DRAM: per-chunk y (two head-group pieces) + gathers ----
    # piece p of chunk c holds local heads {2p, 2p+1}; the AllGather of a
    # piece yields 2048 of the 4096 y-features (host reorders woT rows to
    # match), so each piece can fire as soon as two heads are done and wo
    # accumulates across the two gathered pieces.
    y_dram = [[nc.dram_tensor(f"ych{c}p{p}", [256, CHUNK], BF16)
               for p in range(2)] for c in range(NCH)]
    ag_dram = [[nc.dram_tensor(f"agch{c}p{p}", [DIM // 2, CHUNK], BF16,
                               addr_space="Shared")
                for p in range(2)] for c in range(NCH)]

    with tile.TileContext(nc) as tc:
        pers_cm = tc.tile_pool(name="pers", bufs=1)
        pers = pers_cm.__enter__()
        wqT = pers.tile([128, KT, FQ], BF16, tag="wqT")      # [d, kt, f]
        woT = pers.tile([128, KT, FQ], BF16, tag="woT")      # [yf, kt, of]
        cf = pers.tile([128, SEQ], BF16, tag="cf")           # rope cos [f, t]
        sft = pers.tile([128, SEQ], BF16, tag="sf")          # rope sin [f, t]
        pm = pers.tile([128, 128], BF16, tag="pm")           # signed pairswap
        tri = pers.tile([128, 128], BF16, tag="tri")         # causal 128-blk
        ident = pers.tile([128, 128], BF16, tag="ident")
        kTt = pers.tile([128, SEQ], BF16, tag="kTt")         # [d, t]
        vS = pers.tile([128, TT, HEAD_DIM], BF16, tag="vS")  # [t_loc, tt, dv]
        ones_b = pers.tile([128, 1], BF16, tag="ones_b")

        nc.any.memset(ones_b[:, :], 1.0)

        with tc.tile_pool(name="ps_a", bufs=2, space="PSUM") as pa, \
             tc.tile_pool(name="ps_sc", bufs=3, space="PSUM") as psc, \
             tc.tile_pool(name="ps_o", bufs=2, space="PSUM") as po, \
             tc.tile_pool(name="ps_m", bufs=1, space="PSUM") as pmp, \
             tc.tile_pool(name="xp", bufs=4) as xp, \
             tc.tile_pool(name="wkvp", bufs=4) as wkvp, \
             tc.tile_pool(name="qtp", bufs=2) as qtp, \
             tc.tile_pool(name="rp", bufs=2) as rp, \
             tc.tile_pool(name="sm", bufs=4) as smp, \
             tc.tile_pool(name="exp", bufs=6) as exp_, \
             tc.tile_pool(name="esp", bufs=2) as esp, \
             tc.tile_pool(name="yp", bufs=1) as yp, \
             tc.tile_pool(name="ytp", bufs=2) as ytp, \
             tc.tile_pool(name="owp", bufs=3) as owp:

            def rope_block(src_sb, dst_ap, tslice):
                """dst = src*CF + (Pm.T @ src)*SF  on a [128, CHUNK] block."""
                ps_sw = psc.tile([128, CHUNK], F32, tag="sc")
                nc.tensor.matmul(ps_sw[:, :], pm[:, :], src_sb[:, :],
                                 start=True, stop=True)
                t1 = rp.tile([128, CHUNK], BF16, tag="t1")
                t2 = rp.tile([128, CHUNK], BF16, tag="t2")
                nc.vector.tensor_tensor(out=t1[:, :], in0=src_sb[:, :],
                                        in1=cf[:, tslice], op=AL.mult)
                nc.vector.tensor_tensor(out=t2[:, :], in0=ps_sw[:, :],
                                        in1=sft[:, tslice], op=AL.mult)
                nc.vector.tensor_tensor(out=dst_ap, in0=t1[:, :],
                                        in1=t2[:, :], op=AL.add)

            def emit_wo(c):
                """wo for chunk c from the two gathered head-group pieces.

                Each [128, CHUNK] output accumulates kt 0-15 from piece 0 and
                kt 16-31 from piece 1; the piece-0 half-groups are emitted
                first (piece 0's AllGather completes ~25us before piece 1's),
                two tl groups at a time to fit the 2-bank acc pool. Pieces
                load in 1 MB sub-loads so the first kt tiles land sooner.
                """
                yts = []
                for p in range(2):
                    yt = ytp.tile([128, KT // 2, CHUNK], BF16, tag="yt")
                    for sub in range(2):
                        nc.sync.dma_start(
                            out=yt[:, 8 * sub:8 * (sub + 1), :],
                            in_=ag_dram[c][p].ap()[1024 * sub:
                                                   1024 * (sub + 1), :]
                            .rearrange("(kt p) t -> p kt t", p=128))
                    yts.append(yt)
                for pair in ((0, 1), (2, 3)):
                    pws = []
                    for tl in pair:
                        ps_w = pa.tile([128, CHUNK], F32, tag="acc")
                        for kt in range(16):
                            nc.tensor.matmul(
                                ps_w[:, :],
                                yts[0][:, kt, 128 * tl:128 * (tl + 1)],
                                woT[:, kt, :],
                                start=(kt == 0), stop=False)
                        pws.append(ps_w)
                    for ps_w, tl in zip(pws, pair):
                        for kt in range(16, KT):
                            nc.tensor.matmul(
                                ps_w[:, :],
                                yts[1][:, kt - 16, 128 * tl:128 * (tl + 1)],
                                woT[:, kt, :],
                                start=False, stop=(kt == KT - 1))
                    for ps_w, tl in zip(pws, pair):
                        ow = owp.tile([128, CHUNK], BF16, tag="ow")
                        nc.vector.tensor_copy(out=ow[:, :], in_=ps_w[:, :])
                        r0 = CHUNK * c + 128 * tl
                        nc.gpsimd.dma_start(out=out_ext[r0:r0 + 128, :],
                                            in_=ow[:, :])

            for c in range(NCH):
                tsl = slice(CHUNK * c, CHUNK * (c + 1))
                # ---- loads: 8-ktile grouped DMAs (few large transfers;
                # startup is DMA-issue-rate bound otherwise). x on sync,
                # wkv on gpsimd, wq on scalar. ----
                xgs = []
                wkvgs = []
                for g in range(4):
                    wkvg = wkvp.tile([128, 8, 256], BF16, tag="wkv")
                    nc.gpsimd.dma_start(
                        out=wkvg[:, :, :],
                        in_=wkvT_ext[1024 * g:1024 * (g + 1), :]
                        .rearrange("(kt p) f -> p kt f", p=128))
                    wkvgs.append(wkvg)
                    xg = xp.tile([128, 8, CHUNK], BF16, tag="xT")
                    nc.sync.dma_start(
                        out=xg[:, :, :],
                        in_=x_ext[1024 * g:1024 * (g + 1), tsl]
                        .rearrange("(kt p) t -> p kt t", p=128))
                    xgs.append(xg)

                def xt(k):
                    return xgs[k // 8][:, k % 8, :]

                def wkv(k):
                    return wkvgs[k // 8][:, k % 8, :]
                if c == 0:
                    for k in range(KT):
                        nc.scalar.dma_start(
                            out=wqT[:, k, :],
                            in_=wqT_ext[128 * k:128 * (k + 1), :])
                    nc.scalar.dma_start(out=cf[:, :], in_=cf_ext[:, :])
                    nc.scalar.dma_start(out=sft[:, :], in_=sf_ext[:, :])
                    nc.scalar.dma_start(out=pm[:, :], in_=pm_ext[:, :])
                    nc.scalar.dma_start(out=tri[:, :], in_=tri_ext[:, :])
                    nc.scalar.dma_start(out=ident[:, :], in_=id_ext[:, :])
                    for k in range(KT):
                        nc.scalar.dma_start(
                            out=woT[:, k, :],
                            in_=woT_ext[128 * k:128 * (k + 1), :])

                # ---- projection: weights stationary, xT moving; k/v first --
                ps_k = pa.tile([128, CHUNK], F32, tag="acc")
                for k in range(KT):
                    nc.tensor.matmul(ps_k[:, :], wkv(k)[:, 0:128],
                                     xt(k),
                                     start=(k == 0), stop=(k == KT - 1))
                kb = rp.tile([128, CHUNK], BF16, tag="qb")
                nc.vector.tensor_copy(out=kb[:, :], in_=ps_k[:, :])
                rope_block(kb, kTt[:, tsl], tsl)
                # v -> vT [dv, t], then transpose to natural [t, dv]
                ps_v = pa.tile([128, CHUNK], F32, tag="acc")
                for k in range(KT):
                    nc.tensor.matmul(ps_v[:, :], wkv(k)[:, 128:256],
                                     xt(k),
                                     start=(k == 0), stop=(k == KT - 1))
                vtb = rp.tile([128, CHUNK], BF16, tag="qb")
                nc.vector.tensor_copy(out=vtb[:, :], in_=ps_v[:, :])
                for tl in range(4):
                    ps_tr = pmp.tile([128, 128], BF16, tag="m")
                    nc.tensor.transpose(ps_tr[:, :],
                                        vtb[:, 128 * tl:128 * (tl + 1)],
                                        ident[:, :])
                    nc.vector.tensor_copy(out=vS[:, 4 * c + tl, :],
                                          in_=ps_tr[:, :])
                # q blocks
                qT = qtp.tile([128, HQ, CHUNK], BF16, tag="qT")
                for b in range(HQ):
                    ps_q = pa.tile([128, CHUNK], F32, tag="acc")
                    for k in range(KT):
                        nc.tensor.matmul(ps_q[:, :],
                                         wqT[:, k, 128 * b:128 * (b + 1)],
                                         xt(k),
                                         start=(k == 0), stop=(k == KT - 1))
                    qb = rp.tile([128, CHUNK], BF16, tag="qb")
                    nc.vector.tensor_copy(out=qb[:, :], in_=ps_q[:, :])
                    rope_block(qb, qT[:, b, :], tsl)

                # ---- attention for chunk c ----
                y_sb = yp.tile([128, HQ, CHUNK], BF16, tag="y")
                njt = 4 * (c + 1)
                for h in range(HQ):
                    ps_o = po.tile([128, CHUNK], F32, tag="o")
                    exsum = esp.tile([128, CHUNK], F32, tag="es")
                    for jt in range(njt):
                        p = jt - 4 * c
                        lo = 128 * p if p > 0 else 0
                        ps_s = psc.tile([128, CHUNK], F32, tag="sc")
                        nc.tensor.matmul(ps_s[:, lo:CHUNK],
                                         kTt[:, 128 * jt:128 * (jt + 1)],
                                         qT[:, h, lo:CHUNK],
                                         start=True, stop=True)
                        exf = exp_.tile([128, CHUNK], BF16, tag="ex")
                        nc.scalar.activation(
                            out=exf[:, lo:CHUNK], in_=ps_s[:, lo:CHUNK],
                            func=mybir.ActivationFunctionType.Exp,
                            scale=SCALE)
                        if p >= 0:
                            nc.vector.tensor_tensor(
                                out=exf[:, lo:lo + 128],
                                in0=exf[:, lo:lo + 128],
                                in1=tri[:, :], op=AL.mult)
                        nc.tensor.matmul(ps_o[:, lo:CHUNK], vS[:, jt, :],
                                         exf[:, lo:CHUNK],
                                         start=(jt == 0),
                                         stop=(jt == njt - 1))
                        if jt == 0:
                            nc.vector.tensor_copy(out=exsum[:, :],
                                                  in_=exf[:, :])
                        else:
                            nc.vector.tensor_tensor(
                                out=exsum[:, lo:CHUNK],
                                in0=exsum[:, lo:CHUNK],
                                in1=exf[:, lo:CHUNK], op=AL.add)
                    # softmax denominator -> broadcast 1/l -> normalize
                    esb = rp.tile([128, CHUNK], BF16, tag="esb")
                    nc.vector.tensor_copy(out=esb[:, :], in_=exsum[:, :])
                    ps_l = pmp.tile([1, CHUNK], F32, tag="m")
                    nc.tensor.matmul(ps_l[:, :], ones_b[:, :], esb[:, :],
                                     start=True, stop=True)
                    rr = smp.tile([1, CHUNK], F32, tag="rr")
                    nc.vector.reciprocal_approx_fast(out=rr[:, :],
                                                     in_=ps_l[:, :])
                    bc = rp.tile([128, CHUNK], F32, tag="bc")
                    nc.gpsimd.partition_broadcast(bc[:, :], rr[:, :])
                    nc.vector.tensor_tensor(out=y_sb[:, h, :],
                                            in0=ps_o[:, :], in1=bc[:, :],
                                            op=AL.mult)
                    # ---- piece done after heads 1 and 3: DRAM + AllGather
                    if h % 2 == 1:
                        p = h // 2
                        for hh in (h - 1, h):
                            nc.gpsimd.dma_start(
                                out=y_dram[c][p][128 * (hh - 2 * p):
                                                 128 * (hh - 2 * p + 1), :],
                                in_=y_sb[:, hh, :])
                        nc.gpsimd.collective_compute(
                            "AllGather", AL.bypass,
                            replica_groups=[list(range(N_CORES))],
                            ins=[y_dram[c][p].ap().opt()],
                            outs=[ag_dram[c][p].ap().opt()])
                    # previous chunk's wo fills the rest of this attention
                    # (but keep the last chunk's attention contiguous so its
                    # AllGather pieces fire as early as possible; wo(2) then
                    # fills the wait for them)
                    if h == 1 and 1 <= c <= 2:
                        emit_wo(c - 1)
            emit_wo(NCH - 2)
            emit_wo(NCH - 1)

        pers_cm.__exit__(None, None, None)

    nc.finalize()
    return nc


_NC_CACHE = None


def _get_nc():
    global _NC_CACHE
    if _NC_CACHE is None:
        _NC_CACHE = build_nc()
    return _NC_CACHE


def _host_constants():
    bf = ml_dtypes.bfloat16
    m = np.arange(64, dtype=np.float64)
    freqs = 1.0 / (ROPE_THETA ** (2.0 * m / HEAD_DIM))
    t = np.arange(SEQ, dtype=np.float64)
    ang = np.outer(freqs, t)                                 # [64, SEQ]
    cfv = np.repeat(np.cos(ang), 2, axis=0).astype(bf)       # [128, SEQ]
    sfv = np.repeat(np.sin(ang), 2, axis=0).astype(bf)
    # signed pair swap: out[2m] = -in[2m+1], out[2m+1] = +in[2m]
    # out = Pm.T @ in  ->  Pm[2m+1, 2m] = -1 ; Pm[2m, 2m+1] = +1
    pmv = np.zeros((128, 128), np.float32)
    idx = np.arange(0, 128, 2)
    pmv[idx + 1, idx] = -1.0
    pmv[idx, idx + 1] = 1.0
    pmv = pmv.astype(bf)
    j = np.arange(128)[:, None]
    i = np.arange(128)[None, :]
    triv = (j <= i).astype(np.float32).astype(bf)
    identv = np.eye(128, dtype=bf)
    return cfv, sfv, pmv, triv, identv


def _make_in_maps(x, wq, wk, wv, wo):
    cfv, sfv, pmv, triv, identv = _host_constants()
    bf = ml_dtypes.bfloat16
    xT2 = np.ascontiguousarray(x.reshape(SEQ, DIM).astype(bf).T)
    wqT = np.ascontiguousarray(wq.T.astype(bf))              # [DIM, 4096]
    wkT = wk.T.astype(bf)                                    # [DIM, 1024]
    wvT = wv.T.astype(bf)
    in_maps = []
    for c in range(N_CORES):
        wkvT = np.concatenate([wkT[:, HEAD_DIM * c:HEAD_DIM * (c + 1)],
                               wvT[:, HEAD_DIM * c:HEAD_DIM * (c + 1)]],
                              axis=1)
        # rows of wo for OUR output columns, transposed: [DIM(yfeat), FQ].
        # The AllGather pieces deliver y-features in order
        # [rank0 h01, rank1 h01, ..., rank7 h01, rank0 h23, ...]:
        # reorder woT rows to match.
        woTc = wo[FQ * c:FQ * (c + 1), :].T.astype(bf)       # [DIM, FQ]
        woTc = np.ascontiguousarray(
            woTc.reshape(N_CORES, 2, 256, FQ).transpose(1, 0, 2, 3)
                .reshape(DIM, FQ))
        in_maps.append({
            "xT": xT2,
            "wqT": np.ascontiguousarray(wqT[:, FQ * c:FQ * (c + 1)]),
            "wkvT": np.ascontiguousarray(wkvT),
            "woT": woTc,
            "cf": cfv, "sf": sfv, "pm": pmv, "tri": triv, "ident": identv,
        })
    return in_maps


def _assemble(results):
    # core c holds out[:, 512c:512c+512]
    cols = [np.asarray(results[c]["out"]).astype(np.float32)
            for c in range(N_CORES)]
    return np.concatenate(cols, axis=1).reshape(1, SEQ, DIM)


def run(inputs, trace=False, tmpdir=None):
    nc = _get_nc()
    in_maps = _make_in_maps(inputs["x"], inputs["wq"], inputs["wk"],
                            inputs["wv"], inputs["wo"])
    res = run_bass_kernel_spmd(nc, in_maps, list(range(N_CORES)),
                               trace=trace, tmpdir=tmpdir)
    return _assemble(res.results), res


def kernel(x, start_pos, wq, wk, wv, wo):
    out, _ = run({"x": np.asarray(x), "wq": np.asarray(wq),
                  "wk": np.asarray(wk), "wv": np.asarray(wv),
                  "wo": np.asarray(wo)})
    return out


if __name__ == "__main__":
    rng = np.random.default_rng(0)
    x = rng.standard_normal((1, SEQ, DIM)).astype(np.float32)
    wq = (rng.standard_normal((DIM, DIM)) * DIM ** -0.5).astype(np.float32)
    wk = (rng.standard_normal((1024, DIM)) * DIM ** -0.5).astype(np.float32)
    wv = (rng.standard_normal((1024, DIM)) * DIM ** -0.5).astype(np.float32)
    wo = (rng.standard_normal((DIM, DIM)) * DIM ** -0.5).astype(np.float32)
    out = kernel(x, 0, wq, wk, wv, wo)
    print(out.shape, out.dtype, np.abs(out).mean())
